# revision 36
# baseline (speedup 1.0000x reference)
"""Trainium2 Bass kernel v3 for nn_AttributedEncoder (GNN attribute message passing).

Strategy (8 NeuronCores, SPMD, no collectives):
  - Host does balanced node->(core, window, slot) packing (LPT on node degree),
    49 windows of 128 nodes per core; windows sorted by load so tile counts
    align across cores.  NT = sum_w max_k ceil(load/128) ~ 300 tiles.
  - VAL side: host materializes per-edge-slot val rows (np.take) into a dense
    per-core [NT*128, 256] bf16 table -> device reads them as SEQUENTIAL
    static DMA streams (one 3D DMA per window).  No val gather at all.
  - ATT side: one SWDGE gather per window from a device-built 1.5MB bf16
    table att_ext[2048, 384]: [attW(256) | ones | s_hi | s_lo | garbage].
    (fp8 attW was tried: max-err metric exposes ~6% element error on
    single-edge nodes -> rel err 0.019, too close to the 2e-2 gate.)
  - Aggregation per window w in PSUM: aggA[:,0:257] += S.T @ att_row(fp8,
    ones rider gives rowsum in col 256); aggB += S.T @ val_row(bf16).
    S = onehot(hrel) * exp(lrelu(z_ent + z_att)) in bf16; z_ent via fused
    tensor_tensor_reduce over bf16 one-hot x f32 entb (exact f32 select).
  - Finalize: vcp=copy(aggB) -> dma_start_transpose -> 2 W2 matmuls
    accumulating into aggA; fin = aggA*rr + ent; elu via Relu + min(Exp,1)-1.
  - Phase 0: att scores via tiny f32 PE matmuls from attfT (no attf load),
    batched hi/lo split; att_ext ready in ~20us so gathers start early.
"""
import os
import sys
import types

import numpy as np

sys.path.insert(0, "/opt/trn_rl_repo")
if "/root/.axon_site" not in sys.path:
    sys.path.insert(0, "/root/.axon_site")


def _install_trace_hook():
    try:
        import antenv
        if "antenv.axon_hooks" in sys.modules:
            return
        from trn_agent_boot.trn_boot import _ntff_profile_via_ctypes

        hook = _ntff_profile_via_ctypes("/opt/axon/libaxon_pjrt.so")
        mod = types.ModuleType("antenv.axon_hooks")
        mod.get_axon_ntff_profile_hook = lambda: hook
        mod.set_axon_ntff_profile_hook = lambda h: None
        sys.modules["antenv.axon_hooks"] = mod
        antenv.axon_hooks = mod
    except Exception:
        pass


_install_trace_hook()

from concourse import bass, mybir, tile  # noqa: E402
from concourse import bass_utils as _bu  # noqa: E402
from concourse import library_config  # noqa: E402
from concourse.library_overlay import lower_extended_insts  # noqa: E402
from concourse.masks import make_identity  # noqa: E402
from concourse.tile import add_dep_helper  # noqa: E402
from concourse.mybir import AxisListType, AluOpType, ActivationFunctionType  # noqa: E402

_bu.upload_artifacts = lambda tmpdir: f"file://{tmpdir}"

USE_DMAT = os.environ.get("V3_DMAT", "1") == "1"
USE_TTR = os.environ.get("V3_TTR", "0") == "1"
USE_3D = os.environ.get("V3_3D", "1") == "1"

P = 128
D = 256
N_ENT = 50000
N_ATT = 2000
N_VAL = 100000
E_TOT = 300000
N_CORES = 8
NODES_PER_CORE = 6272          # 49 windows of 128; 8*6272 = 50176 >= 50000
NW = NODES_PER_CORE // P       # 49
NATT_PAD = 2048
AEXT = 384                     # bf16 row: attW(256)|ones|shi|slo|garbage pad
PAD_HREL = 200.0


def legalize_waits(nc, max_engine_waits=1):
    """Hoist excess sync waits onto standalone EventSemaphore instructions on
    the op's own sequencer engine (queue DMAs encode at most one wait)."""
    wid = 0
    for b in nc.m.functions[0].blocks:
        newinsts = []
        for inst in b.instructions:
            si = getattr(inst, "sync_info", None)
            ow = list(si.on_wait) if si and si.on_wait else []
            qname = getattr(inst, "queue", None)
            is_q = bool(qname) or type(inst).__name__ in (
                "InstDMAGatherAnt", "InstDMAScatterAddAnt", "InstDMACopy",
                "InstDmaTransposeAnt", "InstNoOp")
            limit = 1 if is_q else max_engine_waits
            if len(ow) > limit:
                while len(ow) > limit:
                    w, ow = ow[0], ow[1:]
                    es = mybir.InstEventSemaphore(
                        name=f"WAITC-{wid}", engine=inst.engine, ins=[], outs=[])
                    wid += 1
                    es.sync_info = mybir.SyncInfo(on_wait=[w], on_update=[])
                    try:
                        nc.register_instruction(es)
                    except Exception:
                        pass
                    newinsts.append(es)
                si.on_wait = ow
            newinsts.append(inst)
        b.instructions = newinsts


def reassign_swdge_queues(nc, n_queues=4):
    """Tile assigns DMASW sem lanes in final instruction order (mod 8); the
    sim locks each lane to one SWDGE queue. Renumber queue_num in the same
    order so lane L always pairs with queue L % n_queues."""
    cnt = 0
    for b in nc.m.functions[0].blocks:
        for inst in b.instructions:
            if type(inst).__name__ in ("InstDMAGatherAnt", "InstDMAScatterAddAnt"):
                inst.queue_num = cnt % n_queues
                cnt += 1
            elif type(inst).__name__ == "InstDMACopy" and \
                    getattr(inst, "queue", "") == "qPoolDynamic":
                cnt += 1


def _pack16(flat):
    """dma_gather index layout: idxs_ap[p, s] = flat[s*16 + p], first-16-row
    block replicated across the 8 Q7 core groups (128 partitions)."""
    n = len(flat)
    assert n % 16 == 0
    blk = np.asarray(flat, dtype=np.int16).reshape(n // 16, 16).T
    return np.tile(blk, (8, 1))


def _lpt_pack(degrees, n_bins, slot_cap):
    """LPT-pack items (node ids with degrees) into n_bins with <=slot_cap
    items per bin, balancing total degree. Returns list of lists of ids."""
    import heapq
    order = np.argsort(-degrees, kind="stable")
    heap = [(0, i, 0) for i in range(n_bins)]  # (load, bin, count)
    heapq.heapify(heap)
    bins = [[] for _ in range(n_bins)]
    deferred = []
    for nid in order:
        while True:
            load, b, cnt = heapq.heappop(heap)
            if cnt < slot_cap:
                bins[b].append(int(nid))
                heapq.heappush(heap, (load + int(degrees[nid]), b, cnt + 1))
                break
            deferred.append((load, b, cnt))
        for item in deferred:
            heapq.heappush(heap, item)
        deferred.clear()
    return bins


def _host_plan(attribute_triples):
    """Balanced node->(core, window, slot) packing + shared tile schedule.

    Returns plan dict with:
      node_of  [N_CORES, NODES_PER_CORE] int64 node id or -1 pad
      T        [NW] tiles per window (shared)
      NT       total tiles
      edges per (core): per-slot (hrel, val_id, att_id) arrays [NT*128]
    """
    trip = np.asarray(attribute_triples)
    h = trip[:, 0].astype(np.int64)
    val = trip[:, 1].astype(np.int64)
    att = trip[:, 2].astype(np.int64)
    deg = np.bincount(h, minlength=N_ENT)

    core_bins = _lpt_pack(deg, N_CORES, NODES_PER_CORE)
    node_of = np.full((N_CORES, NODES_PER_CORE), -1, dtype=np.int64)
    win_loads = np.zeros((N_CORES, NW), dtype=np.int64)
    for k in range(N_CORES):
        nodes_k = np.array(core_bins[k], dtype=np.int64)
        wbins = _lpt_pack(deg[nodes_k], NW, P)
        # sort windows by load desc so heavy windows share an index across cores
        loads = [int(deg[nodes_k[wb]].sum()) for wb in wbins]
        order = np.argsort([-x for x in loads], kind="stable")
        for wi, wsrc in enumerate(order):
            wb = wbins[wsrc]
            win_loads[k, wi] = loads[wsrc]
            for p, local_i in enumerate(wb):
                node_of[k, wi * P + p] = nodes_k[local_i]

    T = np.maximum(1, -(-win_loads.max(axis=0) // P))  # [NW]
    NT = int(T.sum())
    t0_of_w = np.concatenate([[0], np.cumsum(T)[:-1]])

    # slot assignment per core
    hrel_all = np.full((N_CORES, NT * P), PAD_HREL, dtype=np.float32)
    vid_all = np.zeros((N_CORES, NT * P), dtype=np.int64)
    aid_all = np.zeros((N_CORES, NT * P), dtype=np.int64)
    ecore = np.full(N_ENT, -1, dtype=np.int64)
    erel = np.zeros(N_ENT, dtype=np.int64)   # w*128+p within core
    for k in range(N_CORES):
        m = node_of[k] >= 0
        ecore[node_of[k][m]] = k
        erel[node_of[k][m]] = np.nonzero(m)[0]
    ek = ecore[h]
    erel_e = erel[h]
    for k in range(N_CORES):
        em = ek == k
        rel = erel_e[em]
        w_e = rel // P
        p_e = rel % P
        order = np.argsort(w_e, kind="stable")
        w_s, p_s, v_s, a_s = w_e[order], p_e[order], val[em][order], att[em][order]
        # fill slots: per window, consecutive slots in its tiles
        pos_in_w = np.zeros(len(w_s), dtype=np.int64)
        start = 0
        for w in range(NW):
            cnt = int((w_s == w).sum())
            pos_in_w[start:start + cnt] = np.arange(cnt)
            start += cnt
        slot = (t0_of_w[w_s] * P + pos_in_w)
        assert (pos_in_w < T[w_s] * P).all()
        hrel_all[k, slot] = p_s.astype(np.float32)
        vid_all[k, slot] = v_s
        aid_all[k, slot] = a_s
    return dict(node_of=node_of, T=T, NT=NT, t0=t0_of_w,
                hrel=hrel_all, vid=vid_all, aid=aid_all)


def build_program(T):
    """T: [NW] tiles per window (shared schedule)."""
    NT = int(np.sum(T))
    t0s = np.concatenate([[0], np.cumsum(T)[:-1]])
    nc = bass.Bass(num_swdge_queues=4)
    f32 = mybir.dt.float32
    bf16 = mybir.dt.bfloat16
    fp8 = mybir.dt.float8e4
    ent_d = nc.declare_dram_parameter("ent", [NODES_PER_CORE, D], bf16, isOutput=False)
    attfT_d = nc.declare_dram_parameter("attfT", [D, NATT_PAD], bf16, isOutput=False)
    attf_d = nc.declare_dram_parameter("attf", [NATT_PAD, D], bf16, isOutput=False)
    a2r_d = nc.declare_dram_parameter("a2r", [P, D], bf16, isOutput=False)
    valrows_d = nc.declare_dram_parameter("valrows", [NT * P, D], bf16, isOutput=False)
    a1r_d = nc.declare_dram_parameter("a1r", [P, D], f32, isOutput=False)
    br_d = nc.declare_dram_parameter("br", [P, 1], f32, isOutput=False)
    w1_d = nc.declare_dram_parameter("w1", [D, D], bf16, isOutput=False)
    w2_d = nc.declare_dram_parameter("w2", [D, D], bf16, isOutput=False)
    iota_d = nc.declare_dram_parameter("iota", [P, P], bf16, isOutput=False)
    hrel_d = nc.declare_dram_parameter("hrelf", [P, NT], bf16, isOutput=False)
    aidx_d = nc.declare_dram_parameter("aidx", [P, 8 * NT], mybir.dt.int16, isOutput=False)
    out_d = nc.declare_dram_parameter("out", [NODES_PER_CORE, D], bf16, isOutput=True)
    att_ext = nc.dram_tensor("att_ext", [NATT_PAD, 2 * AEXT], mybir.dt.uint8)
    entscT_d = nc.dram_tensor("entscT_scratch", [NODES_PER_CORE // P, P], f32)

    NCH = NATT_PAD // P  # 16

    with tile.TileContext(nc) as tc:
        nc.gpsimd.load_library(library_config.mlp)
        with (
            tc.tile_pool(name="const", bufs=1) as cp,
            tc.tile_pool(name="ent", bufs=1) as ep,
            tc.tile_pool(name="ph0", bufs=2) as php,
            tc.tile_pool(name="ga", bufs=6) as gap,
            tc.tile_pool(name="gv", bufs=6) as gvp,
            tc.tile_pool(name="s0", bufs=3) as s0p,
            tc.tile_pool(name="junk", bufs=2) as jkp,
            tc.tile_pool(name="ssm", bufs=4) as ssp,
            tc.tile_pool(name="scr", bufs=16) as scp,
            tc.tile_pool(name="entb", bufs=5) as ebp,
            tc.tile_pool(name="fin", bufs=6) as fip,
            tc.tile_pool(name="aggA", bufs=3, space="PSUM") as pA,
            tc.tile_pool(name="aggB", bufs=3, space="PSUM") as pB,
            tc.tile_pool(name="ebpp", bufs=2, space="PSUM") as pE,
        ):
            # ---- constants / parameters to SBUF ----
            # att-table path data FIRST (gates the whole pipeline start)
            attTb = cp.tile([P, 2 * NATT_PAD], bf16, tag="attTb")
            for j in range(2):
                nc.sync.dma_start(out=attTb[:, j * NATT_PAD:(j + 1) * NATT_PAD],
                                  in_=attfT_d[j * P:(j + 1) * P, :])
            w1s = cp.tile([P, 2 * D], bf16, tag="w1s")  # W1 d-chunks
            for j in range(2):
                nc.sync.dma_start(out=w1s[:, j * D:(j + 1) * D],
                                  in_=w1_d[j * P:(j + 1) * P, :])
            a2r = cp.tile([P, D], bf16, tag="a2r")
            nc.sync.dma_start(out=a2r[:], in_=a2r_d[:])
            br = cp.tile([P, 1], f32, tag="br")
            nc.sync.dma_start(out=br[:], in_=br_d[:])
            aidx_s = cp.tile([P, 8 * NT], mybir.dt.int16, tag="aidx")
            nc.sync.dma_start(out=aidx_s[:], in_=aidx_d[:])
            ident = cp.tile([P, P], f32, tag="ident")
            make_identity(nc, ident[:])
            identb = cp.tile([P, P], bf16, tag="identb")
            nc.vector.tensor_copy(out=identb[:], in_=ident[:])
            iota_s = cp.tile([P, P], bf16, tag="iota")
            nc.sync.dma_start(out=iota_s[:], in_=iota_d[:])
            a1r = cp.tile([P, D], f32, tag="a1r")
            nc.sync.dma_start(out=a1r[:], in_=a1r_d[:])
            w2s = cp.tile([P, 2 * D], bf16, tag="w2s")  # W2 f-chunks
            for j in range(2):
                nc.sync.dma_start(out=w2s[:, j * D:(j + 1) * D],
                                  in_=w2_d[j * P:(j + 1) * P, :])
            hrel_s = cp.tile([P, NT], bf16, tag="hrel")
            nc.sync.dma_start(out=hrel_s[:], in_=hrel_d[:])
            zero_t = cp.tile([P, 1], f32, tag="zt")
            nc.vector.memset(zero_t[:], 0.0)
            eps_t = cp.tile([P, 1], f32, tag="et")
            nc.vector.memset(eps_t[:], 1e-30)

            # ---- phase 0b: att_ext = [attW bf16 | ones | score f32 rider] ----
            # att scores on DVE: batched 3D mult+reduce over attf rows
            # (scores land directly in [128 part, 16 chunk-col] layout)
            scol = cp.tile([P, NCH], f32, tag="p0s")
            for c0 in range(0, NCH, 8):
                nb = min(8, NCH - c0)
                afc = php.tile([P, 8, D], bf16, tag="p0af",
                               padded_shape=[P, 8, D])
                nc.sync.dma_start(
                    out=afc[:, 0:nb, :],
                    in_=attf_d[c0 * P:(c0 + nb) * P, :].rearrange(
                        "(c p) d -> p c d", p=P, c=nb))
                jk = php.tile([P, 8, D], f32, tag="p0jk",
                              padded_shape=[P, 8, D])
                nc.vector.tensor_tensor(
                    out=jk[:, 0:nb, :], in0=afc[:, 0:nb, :],
                    in1=a2r[:].unsqueeze(1).to_broadcast([P, nb, D]),
                    op=AluOpType.mult)
                nc.vector.reduce_sum(out=scol[:, c0:c0 + nb],
                                     in_=jk[:, 0:nb, :], axis=AxisListType.X)
            nc.vector.tensor_tensor(out=scol[:], in0=scol[:],
                                    in1=br[:].to_broadcast([P, NCH]),
                                    op=AluOpType.add)
            entres = ep.tile([P, NW * D], bf16, tag="entres")
            entsc = cp.tile([P, NW], f32, tag="entsc")
            EB = 7

            def ent_chunk(w0):
                nb = min(EB, NW - w0)
                esrc = ent_d[w0 * P:(w0 + nb) * P, :].rearrange(
                    "(w p) d -> p w d", p=P, w=nb)
                dst = entres[:, w0 * D:(w0 + nb) * D].rearrange(
                    "p (w d) -> p w d", w=nb, d=D)
                nc.sync.dma_start(out=dst, in_=esrc)
                jk = php.tile([P, EB, D], f32, tag="p0jk",
                              padded_shape=[P, EB, D])
                nc.vector.tensor_tensor(
                    out=jk[:, 0:nb, :],
                    in0=entres[:, w0 * D:(w0 + nb) * D].rearrange(
                        "p (w d) -> p w d", w=nb, d=D),
                    in1=a1r[:].unsqueeze(1).to_broadcast([P, nb, D]),
                    op=AluOpType.mult)
                nc.vector.reduce_sum(out=entsc[:, w0:w0 + nb],
                                     in_=jk[:, 0:nb, :], axis=AxisListType.X)

            ent_chunk(0)
            nc.sync.dma_start(out=entscT_d[0:EB, :].rearrange("w n -> n w"),
                              in_=entsc[:, 0:EB])

            for ch in range(NCH):
                wpsum = pB.tile([P, D], f32, tag="aggB", space="PSUM")
                for j in range(2):
                    mm = nc.tensor.matmul(
                        out=wpsum[:],
                        lhsT=attTb[:, j * NATT_PAD + ch * P: j * NATT_PAD + (ch + 1) * P],
                        rhs=w1s[:, j * D:(j + 1) * D],
                        start=(j == 0), stop=(j == 1))
                    mm.ins.bass_skip_group_check = True
                asm = php.tile([P, 2 * AEXT], mybir.dt.uint8, tag="p0m")
                asmb = asm[:].bitcast(bf16)  # [P, AEXT] bf16 view
                nc.scalar.activation(out=asmb[:, 0:D], in_=wpsum[:],
                                     func=ActivationFunctionType.Copy)
                nc.vector.memset(asmb[:, D:AEXT], 1.0)  # ones col + pad
                nc.vector.tensor_copy(out=asm[:, 516:520].bitcast(f32),
                                      in_=scol[:, ch:ch + 1])
                nc.sync.dma_start(out=att_ext[ch * P:(ch + 1) * P, :],
                                  in_=asm[:])

            # ---- phase 0a: per-window ent scores via PE (chunks of 8 wins,
            # chunk 0 computed FIRST so the pipeline can start immediately) ----
            for w0 in range(EB, NW, EB):
                ent_chunk(w0)
            nc.sync.dma_start(out=entscT_d[EB:, :].rearrange("w n -> n w"),
                              in_=entsc[:, EB:])

            # ---- software-pipelined main loop: A(w) load | B(w-1) compute
            # | C(w-2) finalize ----
            nreg = {}
            for w in range(NW):
                for v in (P * (int(T[w]) // 2), P * (int(T[w]) - int(T[w]) // 2)):
                    if v and v not in nreg:
                        nreg[v] = nc.gpsimd.to_reg(v)

            st = {}   # per-window state

            def stage_load(w):
                tw = int(T[w])
                t0 = int(t0s[w])
                aslot = gap.tile([P, tw, 2 * AEXT], mybir.dt.uint8, tag="ga",
                                 padded_shape=[P, 8, 2 * AEXT])
                h1 = tw // 2
                for (b0, nb) in ((0, h1), (h1, tw - h1)):
                    if nb == 0:
                        continue
                    nc.gpsimd.dma_gather(
                        out_ap=aslot[:, b0:b0 + nb, :], in_ap=att_ext[:],
                        idxs_ap=aidx_s[:, 8 * (t0 + b0):8 * (t0 + b0 + nb)],
                        num_idxs=P * nb, num_idxs_reg=nreg[P * nb],
                        elem_size=2 * AEXT, single_packet=False, queue_num=0)
                vslot = gvp.tile([P, tw, D], bf16, tag="gv",
                                 padded_shape=[P, 8, D])
                vsrc = valrows_d[t0 * P:(t0 + tw) * P, :].rearrange(
                    "(t p) d -> p t d", p=P, t=tw)
                nc.sync.dma_start(out=vslot[:], in_=vsrc)
                entb = ebp.tile([P, P], f32, tag="entb")
                nc.sync.dma_start(
                    out=entb[:],
                    in_=entscT_d[w:w + 1, :].to_broadcast([P, P]))
                st[w] = dict(aslot=aslot, vslot=vslot, entb=entb)

            def stage_scores(w):
                tw = int(T[w])
                t0 = int(t0s[w])
                s = st[w]
                aslot = s["aslot"]
                S0 = s0p.tile([P, tw, P], bf16, tag="S0",
                              padded_shape=[P, 8, P])
                nc.vector.tensor_tensor(
                    out=S0[:],
                    in0=hrel_s[:, t0:t0 + tw].unsqueeze(2)
                    .to_broadcast([P, tw, P]),
                    in1=iota_s[:].unsqueeze(1).to_broadcast([P, tw, P]),
                    op=AluOpType.is_equal)
                entb = s["entb"]
                jk = jkp.tile([P, tw, P], f32, tag="jk",
                              padded_shape=[P, 8, P])
                nc.vector.tensor_tensor(
                    out=jk[:], in0=S0[:],
                    in1=entb[:].unsqueeze(1).to_broadcast([P, tw, P]),
                    op=AluOpType.mult)
                zc = scp.tile([P, tw], f32, tag="zc", padded_shape=[P, 8])
                nc.vector.reduce_sum(out=zc[:], in_=jk[:], axis=AxisListType.X)
                sc_att = aslot[:, :, 516:520].bitcast(f32)  # [P, tw, 1]
                xs = scp.tile([P, tw], f32, tag="xs", padded_shape=[P, 8])
                nc.vector.tensor_tensor(out=xs[:], in0=zc[:],
                                        in1=sc_att[:, :, 0],
                                        op=AluOpType.add)
                lr = scp.tile([P, tw], f32, tag="lr", padded_shape=[P, 8])
                nc.vector.scalar_tensor_tensor(
                    out=lr[:], in0=xs[:], scalar=0.2, in1=xs[:],
                    op0=AluOpType.mult, op1=AluOpType.max)
                sv = scp.tile([P, tw], f32, tag="sv", padded_shape=[P, 8])
                nc.scalar.activation(out=sv[:], in_=lr[:],
                                     func=ActivationFunctionType.Exp)
                Ss = ssp.tile([P, tw, P], bf16, tag="S",
                              padded_shape=[P, 8, P])
                nc.vector.tensor_tensor(
                    out=Ss[:], in0=S0[:],
                    in1=sv[:].unsqueeze(2).to_broadcast([P, tw, P]),
                    op=AluOpType.mult)
                s["Ss"] = Ss

            def stage_agg(w):
                tw = int(T[w])
                s = st[w]
                aslot, vslot, Ss = s["aslot"], s["vslot"], s["Ss"]
                aggA = pA.tile([P, 257], f32, tag="aggA", space="PSUM")
                aggB = pB.tile([P, D], f32, tag="aggB", space="PSUM")
                last_att_mm = None
                for b in range(tw):
                    mma = nc.tensor.matmul(out=aggA[:, 0:257], lhsT=Ss[:, b, :],
                                           rhs=aslot[:, b, 0:514].bitcast(bf16),
                                           start=(b == 0), stop=False)
                    mma.ins.bass_skip_group_check = True
                    last_att_mm = mma
                    mmv = nc.tensor.matmul(out=aggB[:, 0:D], lhsT=Ss[:, b, :],
                                           rhs=vslot[:, b, :],
                                           start=(b == 0), stop=(b == tw - 1))
                    mmv.ins.bass_skip_group_check = True
                s.update(aggA=aggA, aggB=aggB, last_att_mm=last_att_mm)

            def stage_fin1(w):
                s = st[w]
                aggA, aggB = s["aggA"], s["aggB"]
                rr = scp.tile([P, 1], f32, tag="rr")
                nc.vector.tensor_tensor(out=rr[:], in0=aggA[:, 256:257],
                                        in1=eps_t[:], op=AluOpType.max)
                nc.vector.reciprocal(out=rr[:], in_=rr[:])
                vcp = fip.tile([P, D], bf16, tag="vcp")
                nc.scalar.activation(out=vcp[:], in_=aggB[:, 0:D],
                                     func=ActivationFunctionType.Copy)
                tps = fip.tile([P, 2, P], bf16, tag="tps")
                nc.sync.dma_start_transpose(out=tps[:], in_=vcp[:])
                s.update(rr=rr, tps=tps)

            def stage_fin2(w):
                s = st.pop(w)
                aggA = s["aggA"]
                tps, rr = s["tps"], s["rr"]
                for j in range(2):
                    mw = nc.tensor.matmul(out=aggA[:, 0:D],
                                          lhsT=tps[:, j, :],
                                          rhs=w2s[:, j * D:(j + 1) * D],
                                          start=False, stop=(j == 1))
                    mw.ins.bass_skip_group_check = True
                    add_dep_helper(mw.ins, s["last_att_mm"].ins, sync=False,
                                   reason="accumulate after att aggregation")
                fin = fip.tile([P, D], f32, tag="fin")
                nc.scalar.activation(out=fin[:], in_=aggA[:, 0:D],
                                     func=ActivationFunctionType.Copy,
                                     scale=rr[:])
                nc.vector.tensor_add(out=fin[:], in0=fin[:],
                                     in1=entres[:, w * D:(w + 1) * D])
                # elu(x) = relu(x) + (min(exp(x), 1) - 1)
                ex = fip.tile([P, D], f32, tag="ex")
                nc.scalar.activation(out=ex[:], in_=fin[:],
                                     func=ActivationFunctionType.Exp)
                mx = fip.tile([P, D], f32, tag="mx")
                nc.scalar.activation(out=mx[:], in_=fin[:],
                                     func=ActivationFunctionType.Relu)
                exm = fip.tile([P, D], f32, tag="exm")
                nc.vector.tensor_scalar(out=exm[:], in0=ex[:], scalar1=1.0,
                                        scalar2=-1.0, op0=AluOpType.min,
                                        op1=AluOpType.add)
                fob = fip.tile([P, D], bf16, tag="fob")
                nc.vector.tensor_tensor(out=fob[:], in0=exm[:], in1=mx[:],
                                        op=AluOpType.add)
                nc.sync.dma_start(out=out_d[w * P:(w + 1) * P, :], in_=fob[:])

            for i in range(NW + 5):
                if i < NW:
                    stage_load(i)
                if 2 <= i < NW + 2:
                    stage_scores(i - 2)
                if 3 <= i < NW + 3:
                    stage_agg(i - 3)
                if 4 <= i < NW + 4:
                    stage_fin1(i - 4)
                if i >= 5:
                    stage_fin2(i - 5)

    lower_extended_insts(nc)
    reassign_swdge_queues(nc)
    legalize_waits(nc)
    return nc


def _per_core_inputs(plan, att_feats, val_feats, ent_feats, a_w, a_b, W):
    import ml_dtypes
    NT = plan["NT"]
    attf = np.zeros((NATT_PAD, D), dtype=np.float32)
    attf[:N_ATT] = np.asarray(att_feats, dtype=np.float32)
    attfT = np.ascontiguousarray(attf.T).astype(ml_dtypes.bfloat16)
    attfb = attf.astype(ml_dtypes.bfloat16)
    valf = np.asarray(val_feats, dtype=np.float32).astype(ml_dtypes.bfloat16)
    a_w = np.asarray(a_w, dtype=np.float32)
    a1r = np.tile(a_w[0, :D][None, :], (P, 1)).astype(np.float32)
    a2r = np.tile(a_w[0, D:][None, :], (P, 1)).astype(ml_dtypes.bfloat16)
    br = np.full((P, 1), float(np.asarray(a_b).reshape(-1)[0]), dtype=np.float32)
    Wf = np.asarray(W, dtype=np.float32)
    w1 = Wf[:D].astype(ml_dtypes.bfloat16)    # [256, 256] att half
    w2 = Wf[D:].astype(ml_dtypes.bfloat16)    # [256, 256] val half
    iota = np.tile(np.arange(P, dtype=np.float32)[None, :], (P, 1)) \
        .astype(ml_dtypes.bfloat16)
    entf = np.asarray(ent_feats, dtype=np.float32)

    in_maps = []
    for k in range(N_CORES):
        nids = plan["node_of"][k]
        entp = np.zeros((NODES_PER_CORE, D), dtype=ml_dtypes.bfloat16)
        m = nids >= 0
        entp[m] = entf[nids[m]].astype(ml_dtypes.bfloat16)
        valrows = valf[plan["vid"][k]]                       # [NT*128, 256]
        hrelf = plan["hrel"][k].reshape(NT, P)               # [NT, 128]
        aidx = _pack16(plan["aid"][k])                       # [128, 8*NT]
        in_maps.append(dict(
            ent=entp, attfT=attfT, attf=attfb,
            valrows=np.ascontiguousarray(valrows),
            a1r=a1r, a2r=a2r, br=br, w1=w1, w2=w2, iota=iota,
            hrelf=np.ascontiguousarray(hrelf.T).astype(ml_dtypes.bfloat16),
            aidx=aidx,
        ))
    return in_maps


def kernel(attribute_triples, att_feats, val_feats, ent_feats, a_w, a_b, W):
    plan = _host_plan(attribute_triples)
    nc = build_program(plan["T"])
    in_maps = _per_core_inputs(plan, att_feats, val_feats, ent_feats,
                               a_w, a_b, W)
    trace = os.environ.get("KERNEL_TRACE", "0") == "1"
    res = _bu.run_bass_kernel_spmd(nc, in_maps, list(range(N_CORES)), trace=trace)
    if trace and res.exec_time_ns:
        print(f"HW exec time: {res.exec_time_ns} ns")
    out = np.zeros((N_ENT, D), dtype=np.float32)
    for k in range(N_CORES):
        nids = plan["node_of"][k]
        m = nids >= 0
        out[nids[m]] = res.results[k]["out"][m].astype(np.float32)
    return out


# revision 38
# speedup vs baseline: 1.0234x; 1.0234x over previous
"""Trainium2 Bass kernel v3 for nn_AttributedEncoder (GNN attribute message passing).

Strategy (8 NeuronCores, SPMD, no collectives):
  - Host does balanced node->(core, window, slot) packing (LPT on node degree),
    49 windows of 128 nodes per core; windows sorted by load so tile counts
    align across cores.  NT = sum_w max_k ceil(load/128) ~ 300 tiles.
  - VAL side: host materializes per-edge-slot val rows (np.take) into a dense
    per-core [NT*128, 256] bf16 table -> device reads them as SEQUENTIAL
    static DMA streams (one 3D DMA per window).  No val gather at all.
  - ATT side: one SWDGE gather per window from a device-built 1.5MB bf16
    table att_ext[2048, 384]: [attW(256) | ones | s_hi | s_lo | garbage].
    (fp8 attW was tried: max-err metric exposes ~6% element error on
    single-edge nodes -> rel err 0.019, too close to the 2e-2 gate.)
  - Aggregation per window w in PSUM: aggA[:,0:257] += S.T @ att_row(fp8,
    ones rider gives rowsum in col 256); aggB += S.T @ val_row(bf16).
    S = onehot(hrel) * exp(lrelu(z_ent + z_att)) in bf16; z_ent via fused
    tensor_tensor_reduce over bf16 one-hot x f32 entb (exact f32 select).
  - Finalize: vcp=copy(aggB) -> dma_start_transpose -> 2 W2 matmuls
    accumulating into aggA; fin = aggA*rr + ent; elu via Relu + min(Exp,1)-1.
  - Phase 0: att scores via tiny f32 PE matmuls from attfT (no attf load),
    batched hi/lo split; att_ext ready in ~20us so gathers start early.
"""
import os
import sys
import types

import numpy as np

sys.path.insert(0, "/opt/trn_rl_repo")
if "/root/.axon_site" not in sys.path:
    sys.path.insert(0, "/root/.axon_site")


def _install_trace_hook():
    try:
        import antenv
        if "antenv.axon_hooks" in sys.modules:
            return
        from trn_agent_boot.trn_boot import _ntff_profile_via_ctypes

        hook = _ntff_profile_via_ctypes("/opt/axon/libaxon_pjrt.so")
        mod = types.ModuleType("antenv.axon_hooks")
        mod.get_axon_ntff_profile_hook = lambda: hook
        mod.set_axon_ntff_profile_hook = lambda h: None
        sys.modules["antenv.axon_hooks"] = mod
        antenv.axon_hooks = mod
    except Exception:
        pass


_install_trace_hook()

from concourse import bass, mybir, tile  # noqa: E402
from concourse import bass_utils as _bu  # noqa: E402
from concourse import library_config  # noqa: E402
from concourse.library_overlay import lower_extended_insts  # noqa: E402
from concourse.masks import make_identity  # noqa: E402
from concourse.tile import add_dep_helper  # noqa: E402
from concourse.mybir import AxisListType, AluOpType, ActivationFunctionType  # noqa: E402

_bu.upload_artifacts = lambda tmpdir: f"file://{tmpdir}"

USE_DMAT = os.environ.get("V3_DMAT", "1") == "1"
USE_TTR = os.environ.get("V3_TTR", "0") == "1"
USE_3D = os.environ.get("V3_3D", "1") == "1"

P = 128
D = 256
N_ENT = 50000
N_ATT = 2000
N_VAL = 100000
E_TOT = 300000
N_CORES = 8
NODES_PER_CORE = 6272          # 49 windows of 128; 8*6272 = 50176 >= 50000
NW = NODES_PER_CORE // P       # 49
NATT_PAD = 2048
AEXT = 384                     # bf16 row: attW(256)|ones|shi|slo|garbage pad
PAD_HREL = 200.0


def legalize_waits(nc, max_engine_waits=1):
    """Hoist excess sync waits onto standalone EventSemaphore instructions on
    the op's own sequencer engine (queue DMAs encode at most one wait)."""
    wid = 0
    for b in nc.m.functions[0].blocks:
        newinsts = []
        for inst in b.instructions:
            si = getattr(inst, "sync_info", None)
            ow = list(si.on_wait) if si and si.on_wait else []
            qname = getattr(inst, "queue", None)
            is_q = bool(qname) or type(inst).__name__ in (
                "InstDMAGatherAnt", "InstDMAScatterAddAnt", "InstDMACopy",
                "InstDmaTransposeAnt", "InstNoOp")
            limit = 1 if is_q else max_engine_waits
            if len(ow) > limit:
                while len(ow) > limit:
                    w, ow = ow[0], ow[1:]
                    es = mybir.InstEventSemaphore(
                        name=f"WAITC-{wid}", engine=inst.engine, ins=[], outs=[])
                    wid += 1
                    es.sync_info = mybir.SyncInfo(on_wait=[w], on_update=[])
                    try:
                        nc.register_instruction(es)
                    except Exception:
                        pass
                    newinsts.append(es)
                si.on_wait = ow
            newinsts.append(inst)
        b.instructions = newinsts


def reassign_swdge_queues(nc, n_queues=4):
    """Tile assigns DMASW sem lanes in final instruction order (mod 8); the
    sim locks each lane to one SWDGE queue. Renumber queue_num in the same
    order so lane L always pairs with queue L % n_queues."""
    cnt = 0
    for b in nc.m.functions[0].blocks:
        for inst in b.instructions:
            if type(inst).__name__ in ("InstDMAGatherAnt", "InstDMAScatterAddAnt"):
                inst.queue_num = cnt % n_queues
                cnt += 1
            elif type(inst).__name__ == "InstDMACopy" and \
                    getattr(inst, "queue", "") == "qPoolDynamic":
                cnt += 1


def _pack16(flat):
    """dma_gather index layout: idxs_ap[p, s] = flat[s*16 + p], first-16-row
    block replicated across the 8 Q7 core groups (128 partitions)."""
    n = len(flat)
    assert n % 16 == 0
    blk = np.asarray(flat, dtype=np.int16).reshape(n // 16, 16).T
    return np.tile(blk, (8, 1))


def _lpt_pack(degrees, n_bins, slot_cap):
    """LPT-pack items (node ids with degrees) into n_bins with <=slot_cap
    items per bin, balancing total degree. Returns list of lists of ids."""
    import heapq
    order = np.argsort(-degrees, kind="stable")
    heap = [(0, i, 0) for i in range(n_bins)]  # (load, bin, count)
    heapq.heapify(heap)
    bins = [[] for _ in range(n_bins)]
    deferred = []
    for nid in order:
        while True:
            load, b, cnt = heapq.heappop(heap)
            if cnt < slot_cap:
                bins[b].append(int(nid))
                heapq.heappush(heap, (load + int(degrees[nid]), b, cnt + 1))
                break
            deferred.append((load, b, cnt))
        for item in deferred:
            heapq.heappush(heap, item)
        deferred.clear()
    return bins


def _host_plan(attribute_triples):
    """Balanced node->(core, window, slot) packing + shared tile schedule.

    Returns plan dict with:
      node_of  [N_CORES, NODES_PER_CORE] int64 node id or -1 pad
      T        [NW] tiles per window (shared)
      NT       total tiles
      edges per (core): per-slot (hrel, val_id, att_id) arrays [NT*128]
    """
    trip = np.asarray(attribute_triples)
    h = trip[:, 0].astype(np.int64)
    val = trip[:, 1].astype(np.int64)
    att = trip[:, 2].astype(np.int64)
    deg = np.bincount(h, minlength=N_ENT)

    core_bins = _lpt_pack(deg, N_CORES, NODES_PER_CORE)
    node_of = np.full((N_CORES, NODES_PER_CORE), -1, dtype=np.int64)
    win_loads = np.zeros((N_CORES, NW), dtype=np.int64)
    for k in range(N_CORES):
        nodes_k = np.array(core_bins[k], dtype=np.int64)
        wbins = _lpt_pack(deg[nodes_k], NW, P)
        # sort windows by load desc so heavy windows share an index across cores
        loads = [int(deg[nodes_k[wb]].sum()) for wb in wbins]
        order = np.argsort([-x for x in loads], kind="stable")
        for wi, wsrc in enumerate(order):
            wb = wbins[wsrc]
            win_loads[k, wi] = loads[wsrc]
            for p, local_i in enumerate(wb):
                node_of[k, wi * P + p] = nodes_k[local_i]

    T = np.maximum(1, -(-win_loads.max(axis=0) // P))  # [NW]
    NT = int(T.sum())
    t0_of_w = np.concatenate([[0], np.cumsum(T)[:-1]])

    # slot assignment per core
    hrel_all = np.full((N_CORES, NT * P), PAD_HREL, dtype=np.float32)
    vid_all = np.zeros((N_CORES, NT * P), dtype=np.int64)
    aid_all = np.zeros((N_CORES, NT * P), dtype=np.int64)
    ecore = np.full(N_ENT, -1, dtype=np.int64)
    erel = np.zeros(N_ENT, dtype=np.int64)   # w*128+p within core
    for k in range(N_CORES):
        m = node_of[k] >= 0
        ecore[node_of[k][m]] = k
        erel[node_of[k][m]] = np.nonzero(m)[0]
    ek = ecore[h]
    erel_e = erel[h]
    for k in range(N_CORES):
        em = ek == k
        rel = erel_e[em]
        w_e = rel // P
        p_e = rel % P
        order = np.argsort(w_e, kind="stable")
        w_s, p_s, v_s, a_s = w_e[order], p_e[order], val[em][order], att[em][order]
        # fill slots: per window, consecutive slots in its tiles
        pos_in_w = np.zeros(len(w_s), dtype=np.int64)
        start = 0
        for w in range(NW):
            cnt = int((w_s == w).sum())
            pos_in_w[start:start + cnt] = np.arange(cnt)
            start += cnt
        slot = (t0_of_w[w_s] * P + pos_in_w)
        assert (pos_in_w < T[w_s] * P).all()
        hrel_all[k, slot] = p_s.astype(np.float32)
        vid_all[k, slot] = v_s
        aid_all[k, slot] = a_s
    return dict(node_of=node_of, T=T, NT=NT, t0=t0_of_w,
                hrel=hrel_all, vid=vid_all, aid=aid_all)


def build_program(T):
    """T: [NW] tiles per window (shared schedule)."""
    NT = int(np.sum(T))
    t0s = np.concatenate([[0], np.cumsum(T)[:-1]])
    nc = bass.Bass(num_swdge_queues=4)
    f32 = mybir.dt.float32
    bf16 = mybir.dt.bfloat16
    fp8 = mybir.dt.float8e4
    ent_d = nc.declare_dram_parameter("ent", [NODES_PER_CORE, D], bf16, isOutput=False)
    attfT_d = nc.declare_dram_parameter("attfT", [D, NATT_PAD], bf16, isOutput=False)
    attf_d = nc.declare_dram_parameter("attf", [NATT_PAD, D], bf16, isOutput=False)
    a2r_d = nc.declare_dram_parameter("a2r", [P, D], bf16, isOutput=False)
    valrows_d = nc.declare_dram_parameter("valrows", [NT * P, D], bf16, isOutput=False)
    a1r_d = nc.declare_dram_parameter("a1r", [P, D], f32, isOutput=False)
    br_d = nc.declare_dram_parameter("br", [P, 1], f32, isOutput=False)
    w1_d = nc.declare_dram_parameter("w1", [D, D], bf16, isOutput=False)
    w2_d = nc.declare_dram_parameter("w2", [D, D], bf16, isOutput=False)
    iota_d = nc.declare_dram_parameter("iota", [P, P], bf16, isOutput=False)
    hrel_d = nc.declare_dram_parameter("hrelf", [P, NT], bf16, isOutput=False)
    aidx_d = nc.declare_dram_parameter("aidx", [P, 8 * NT], mybir.dt.int16, isOutput=False)
    out_d = nc.declare_dram_parameter("out", [NODES_PER_CORE, D], bf16, isOutput=True)
    att_ext = nc.dram_tensor("att_ext", [NATT_PAD, 2 * AEXT], mybir.dt.uint8)
    entscT_d = nc.dram_tensor("entscT_scratch", [NODES_PER_CORE // P, P], f32)

    NCH = NATT_PAD // P  # 16

    with tile.TileContext(nc) as tc:
        nc.gpsimd.load_library(library_config.mlp)
        with (
            tc.tile_pool(name="const", bufs=1) as cp,
            tc.tile_pool(name="ent", bufs=1) as ep,
            tc.tile_pool(name="ph0", bufs=2) as php,
            tc.tile_pool(name="ga", bufs=6) as gap,
            tc.tile_pool(name="gv", bufs=6) as gvp,
            tc.tile_pool(name="s0", bufs=3) as s0p,
            tc.tile_pool(name="junk", bufs=2) as jkp,
            tc.tile_pool(name="ssm", bufs=4) as ssp,
            tc.tile_pool(name="scr", bufs=16) as scp,
            tc.tile_pool(name="entb", bufs=5) as ebp,
            tc.tile_pool(name="fin", bufs=6) as fip,
            tc.tile_pool(name="aggA", bufs=3, space="PSUM") as pA,
            tc.tile_pool(name="aggB", bufs=3, space="PSUM") as pB,
            tc.tile_pool(name="ebpp", bufs=2, space="PSUM") as pE,
        ):
            # ---- constants / parameters to SBUF ----
            # att-table path data FIRST (gates the whole pipeline start)
            attTb = cp.tile([P, 2 * NATT_PAD], bf16, tag="attTb")
            for j in range(2):
                nc.sync.dma_start(out=attTb[:, j * NATT_PAD:(j + 1) * NATT_PAD],
                                  in_=attfT_d[j * P:(j + 1) * P, :])
            w1s = cp.tile([P, 2 * D], bf16, tag="w1s")  # W1 d-chunks
            for j in range(2):
                nc.sync.dma_start(out=w1s[:, j * D:(j + 1) * D],
                                  in_=w1_d[j * P:(j + 1) * P, :])
            a2r = cp.tile([P, D], bf16, tag="a2r")
            nc.sync.dma_start(out=a2r[:], in_=a2r_d[:])
            br = cp.tile([P, 1], f32, tag="br")
            nc.sync.dma_start(out=br[:], in_=br_d[:])
            aidx_s = cp.tile([P, 8 * NT], mybir.dt.int16, tag="aidx")
            nc.sync.dma_start(out=aidx_s[:], in_=aidx_d[:])
            ident = cp.tile([P, P], f32, tag="ident")
            make_identity(nc, ident[:])
            identb = cp.tile([P, P], bf16, tag="identb")
            nc.vector.tensor_copy(out=identb[:], in_=ident[:])
            iota_s = cp.tile([P, P], bf16, tag="iota")
            nc.sync.dma_start(out=iota_s[:], in_=iota_d[:])
            a1r = cp.tile([P, D], f32, tag="a1r")
            nc.sync.dma_start(out=a1r[:], in_=a1r_d[:])
            w2s = cp.tile([P, 2 * D], bf16, tag="w2s")  # W2 f-chunks
            for j in range(2):
                nc.sync.dma_start(out=w2s[:, j * D:(j + 1) * D],
                                  in_=w2_d[j * P:(j + 1) * P, :])
            hrel_s = cp.tile([P, NT], bf16, tag="hrel")
            nc.sync.dma_start(out=hrel_s[:], in_=hrel_d[:])
            zero_t = cp.tile([P, 1], f32, tag="zt")
            nc.vector.memset(zero_t[:], 0.0)
            onesb = cp.tile([P, 1], bf16, tag="onesb")
            nc.vector.memset(onesb[:], 1.0)
            eps_t = cp.tile([P, 1], f32, tag="et")
            nc.vector.memset(eps_t[:], 1e-30)

            # ---- phase 0b: att_ext = [attW bf16 | ones | score f32 rider] ----
            # att scores on DVE: batched 3D mult+reduce over attf rows
            # (scores land directly in [128 part, 16 chunk-col] layout)
            scol = cp.tile([P, NCH], f32, tag="p0s")
            for c0 in range(0, NCH, 8):
                nb = min(8, NCH - c0)
                afc = php.tile([P, 8, D], bf16, tag="p0af",
                               padded_shape=[P, 8, D])
                nc.sync.dma_start(
                    out=afc[:, 0:nb, :],
                    in_=attf_d[c0 * P:(c0 + nb) * P, :].rearrange(
                        "(c p) d -> p c d", p=P, c=nb))
                jk = php.tile([P, 8, D], f32, tag="p0jk",
                              padded_shape=[P, 8, D])
                nc.vector.tensor_tensor(
                    out=jk[:, 0:nb, :], in0=afc[:, 0:nb, :],
                    in1=a2r[:].unsqueeze(1).to_broadcast([P, nb, D]),
                    op=AluOpType.mult)
                nc.vector.reduce_sum(out=scol[:, c0:c0 + nb],
                                     in_=jk[:, 0:nb, :], axis=AxisListType.X)
            nc.vector.tensor_tensor(out=scol[:], in0=scol[:],
                                    in1=br[:].to_broadcast([P, NCH]),
                                    op=AluOpType.add)
            entres = ep.tile([P, NW * D], bf16, tag="entres")
            entsc = cp.tile([P, NW], f32, tag="entsc")
            EB = 7

            def ent_chunk(w0):
                nb = min(EB, NW - w0)
                esrc = ent_d[w0 * P:(w0 + nb) * P, :].rearrange(
                    "(w p) d -> p w d", p=P, w=nb)
                dst = entres[:, w0 * D:(w0 + nb) * D].rearrange(
                    "p (w d) -> p w d", w=nb, d=D)
                nc.sync.dma_start(out=dst, in_=esrc)
                jk = php.tile([P, EB, D], f32, tag="p0jk",
                              padded_shape=[P, EB, D])
                nc.vector.tensor_tensor(
                    out=jk[:, 0:nb, :],
                    in0=entres[:, w0 * D:(w0 + nb) * D].rearrange(
                        "p (w d) -> p w d", w=nb, d=D),
                    in1=a1r[:].unsqueeze(1).to_broadcast([P, nb, D]),
                    op=AluOpType.mult)
                nc.vector.reduce_sum(out=entsc[:, w0:w0 + nb],
                                     in_=jk[:, 0:nb, :], axis=AxisListType.X)

            ent_chunk(0)
            nc.sync.dma_start(out=entscT_d[0:EB, :].rearrange("w n -> n w"),
                              in_=entsc[:, 0:EB])

            for ch in range(NCH):
                wpsum = pB.tile([P, D], f32, tag="aggB", space="PSUM")
                for j in range(2):
                    mm = nc.tensor.matmul(
                        out=wpsum[:],
                        lhsT=attTb[:, j * NATT_PAD + ch * P: j * NATT_PAD + (ch + 1) * P],
                        rhs=w1s[:, j * D:(j + 1) * D],
                        start=(j == 0), stop=(j == 1))
                    mm.ins.bass_skip_group_check = True
                asm = php.tile([P, 2 * AEXT], mybir.dt.uint8, tag="p0m")
                asmb = asm[:].bitcast(bf16)  # [P, AEXT] bf16 view
                nc.scalar.activation(out=asmb[:, 0:D], in_=wpsum[:],
                                     func=ActivationFunctionType.Copy)
                nc.scalar.activation(
                    out=asmb[:, D:AEXT],
                    in_=onesb[:, 0:1].to_broadcast([P, AEXT - D]),
                    func=ActivationFunctionType.Copy)
                nc.scalar.activation(out=asm[:, 516:520].bitcast(f32),
                                     in_=scol[:, ch:ch + 1],
                                     func=ActivationFunctionType.Copy)
                nc.sync.dma_start(out=att_ext[ch * P:(ch + 1) * P, :],
                                  in_=asm[:])

            # ---- phase 0a: per-window ent scores via PE (chunks of 8 wins,
            # chunk 0 computed FIRST so the pipeline can start immediately) ----
            for w0 in range(EB, NW, EB):
                ent_chunk(w0)
            nc.sync.dma_start(out=entscT_d[EB:, :].rearrange("w n -> n w"),
                              in_=entsc[:, EB:])

            # ---- software-pipelined main loop: A(w) load | B(w-1) compute
            # | C(w-2) finalize ----
            nreg = {}
            for w in range(NW):
                for v in (P * (int(T[w]) // 2), P * (int(T[w]) - int(T[w]) // 2)):
                    if v and v not in nreg:
                        nreg[v] = nc.gpsimd.to_reg(v)

            st = {}   # per-window state

            def stage_load(w):
                tw = int(T[w])
                t0 = int(t0s[w])
                aslot = gap.tile([P, tw, 2 * AEXT], mybir.dt.uint8, tag="ga",
                                 padded_shape=[P, 8, 2 * AEXT])
                h1 = tw // 2
                for (b0, nb) in ((0, h1), (h1, tw - h1)):
                    if nb == 0:
                        continue
                    nc.gpsimd.dma_gather(
                        out_ap=aslot[:, b0:b0 + nb, :], in_ap=att_ext[:],
                        idxs_ap=aidx_s[:, 8 * (t0 + b0):8 * (t0 + b0 + nb)],
                        num_idxs=P * nb, num_idxs_reg=nreg[P * nb],
                        elem_size=2 * AEXT, single_packet=False, queue_num=0)
                vslot = gvp.tile([P, tw, D], bf16, tag="gv",
                                 padded_shape=[P, 8, D])
                vsrc = valrows_d[t0 * P:(t0 + tw) * P, :].rearrange(
                    "(t p) d -> p t d", p=P, t=tw)
                nc.sync.dma_start(out=vslot[:], in_=vsrc)
                entb = ebp.tile([P, P], f32, tag="entb")
                nc.sync.dma_start(
                    out=entb[:],
                    in_=entscT_d[w:w + 1, :].to_broadcast([P, P]))
                st[w] = dict(aslot=aslot, vslot=vslot, entb=entb)

            def stage_scores(w):
                tw = int(T[w])
                t0 = int(t0s[w])
                s = st[w]
                aslot = s["aslot"]
                S0 = s0p.tile([P, tw, P], bf16, tag="S0",
                              padded_shape=[P, 8, P])
                nc.vector.tensor_tensor(
                    out=S0[:],
                    in0=hrel_s[:, t0:t0 + tw].unsqueeze(2)
                    .to_broadcast([P, tw, P]),
                    in1=iota_s[:].unsqueeze(1).to_broadcast([P, tw, P]),
                    op=AluOpType.is_equal)
                entb = s["entb"]
                jk = jkp.tile([P, tw, P], f32, tag="jk",
                              padded_shape=[P, 8, P])
                nc.vector.tensor_tensor(
                    out=jk[:], in0=S0[:],
                    in1=entb[:].unsqueeze(1).to_broadcast([P, tw, P]),
                    op=AluOpType.mult)
                zc = scp.tile([P, tw], f32, tag="zc", padded_shape=[P, 8])
                nc.vector.reduce_sum(out=zc[:], in_=jk[:], axis=AxisListType.X)
                sc_att = aslot[:, :, 516:520].bitcast(f32)  # [P, tw, 1]
                xs = scp.tile([P, tw], f32, tag="xs", padded_shape=[P, 8])
                nc.vector.tensor_tensor(out=xs[:], in0=zc[:],
                                        in1=sc_att[:, :, 0],
                                        op=AluOpType.add)
                lr = scp.tile([P, tw], f32, tag="lr", padded_shape=[P, 8])
                nc.vector.scalar_tensor_tensor(
                    out=lr[:], in0=xs[:], scalar=0.2, in1=xs[:],
                    op0=AluOpType.mult, op1=AluOpType.max)
                sv = scp.tile([P, tw], f32, tag="sv", padded_shape=[P, 8])
                nc.scalar.activation(out=sv[:], in_=lr[:],
                                     func=ActivationFunctionType.Exp)
                Ss = ssp.tile([P, tw, P], bf16, tag="S",
                              padded_shape=[P, 8, P])
                nc.vector.tensor_tensor(
                    out=Ss[:], in0=S0[:],
                    in1=sv[:].unsqueeze(2).to_broadcast([P, tw, P]),
                    op=AluOpType.mult)
                s["Ss"] = Ss

            def stage_agg(w):
                tw = int(T[w])
                s = st[w]
                aslot, vslot, Ss = s["aslot"], s["vslot"], s["Ss"]
                aggA = pA.tile([P, 257], f32, tag="aggA", space="PSUM")
                aggB = pB.tile([P, D], f32, tag="aggB", space="PSUM")
                last_att_mm = None
                for b in range(tw):
                    mma = nc.tensor.matmul(out=aggA[:, 0:257], lhsT=Ss[:, b, :],
                                           rhs=aslot[:, b, 0:514].bitcast(bf16),
                                           start=(b == 0), stop=False)
                    mma.ins.bass_skip_group_check = True
                    last_att_mm = mma
                    mmv = nc.tensor.matmul(out=aggB[:, 0:D], lhsT=Ss[:, b, :],
                                           rhs=vslot[:, b, :],
                                           start=(b == 0), stop=(b == tw - 1))
                    mmv.ins.bass_skip_group_check = True
                s.update(aggA=aggA, aggB=aggB, last_att_mm=last_att_mm)

            def stage_fin1(w):
                s = st[w]
                aggA, aggB = s["aggA"], s["aggB"]
                rr = scp.tile([P, 1], f32, tag="rr")
                nc.vector.tensor_tensor(out=rr[:], in0=aggA[:, 256:257],
                                        in1=eps_t[:], op=AluOpType.max)
                nc.vector.reciprocal(out=rr[:], in_=rr[:])
                vcp = fip.tile([P, D], bf16, tag="vcp")
                nc.scalar.activation(out=vcp[:], in_=aggB[:, 0:D],
                                     func=ActivationFunctionType.Copy)
                tps = fip.tile([P, 2, P], bf16, tag="tps")
                nc.sync.dma_start_transpose(out=tps[:], in_=vcp[:])
                s.update(rr=rr, tps=tps)

            def stage_fin2(w):
                s = st.pop(w)
                aggA = s["aggA"]
                tps, rr = s["tps"], s["rr"]
                for j in range(2):
                    mw = nc.tensor.matmul(out=aggA[:, 0:D],
                                          lhsT=tps[:, j, :],
                                          rhs=w2s[:, j * D:(j + 1) * D],
                                          start=False, stop=(j == 1))
                    mw.ins.bass_skip_group_check = True
                    add_dep_helper(mw.ins, s["last_att_mm"].ins, sync=False,
                                   reason="accumulate after att aggregation")
                fin = fip.tile([P, D], f32, tag="fin")
                nc.scalar.activation(out=fin[:], in_=aggA[:, 0:D],
                                     func=ActivationFunctionType.Copy,
                                     scale=rr[:])
                nc.vector.tensor_add(out=fin[:], in0=fin[:],
                                     in1=entres[:, w * D:(w + 1) * D])
                # elu(x) = relu(x) + (min(exp(x), 1) - 1)
                ex = fip.tile([P, D], f32, tag="ex")
                nc.scalar.activation(out=ex[:], in_=fin[:],
                                     func=ActivationFunctionType.Exp)
                mx = fip.tile([P, D], f32, tag="mx")
                nc.scalar.activation(out=mx[:], in_=fin[:],
                                     func=ActivationFunctionType.Relu)
                exm = fip.tile([P, D], f32, tag="exm")
                nc.vector.tensor_scalar(out=exm[:], in0=ex[:], scalar1=1.0,
                                        scalar2=-1.0, op0=AluOpType.min,
                                        op1=AluOpType.add)
                fob = fip.tile([P, D], bf16, tag="fob")
                nc.vector.tensor_tensor(out=fob[:], in0=exm[:], in1=mx[:],
                                        op=AluOpType.add)
                nc.sync.dma_start(out=out_d[w * P:(w + 1) * P, :], in_=fob[:])

            for i in range(NW + 5):
                if i < NW:
                    stage_load(i)
                if 2 <= i < NW + 2:
                    stage_scores(i - 2)
                if 3 <= i < NW + 3:
                    stage_agg(i - 3)
                if 4 <= i < NW + 4:
                    stage_fin1(i - 4)
                if i >= 5:
                    stage_fin2(i - 5)

    lower_extended_insts(nc)
    reassign_swdge_queues(nc)
    legalize_waits(nc)
    return nc


def _per_core_inputs(plan, att_feats, val_feats, ent_feats, a_w, a_b, W):
    import ml_dtypes
    NT = plan["NT"]
    attf = np.zeros((NATT_PAD, D), dtype=np.float32)
    attf[:N_ATT] = np.asarray(att_feats, dtype=np.float32)
    attfT = np.ascontiguousarray(attf.T).astype(ml_dtypes.bfloat16)
    attfb = attf.astype(ml_dtypes.bfloat16)
    valf = np.asarray(val_feats, dtype=np.float32).astype(ml_dtypes.bfloat16)
    a_w = np.asarray(a_w, dtype=np.float32)
    a1r = np.tile(a_w[0, :D][None, :], (P, 1)).astype(np.float32)
    a2r = np.tile(a_w[0, D:][None, :], (P, 1)).astype(ml_dtypes.bfloat16)
    br = np.full((P, 1), float(np.asarray(a_b).reshape(-1)[0]), dtype=np.float32)
    Wf = np.asarray(W, dtype=np.float32)
    w1 = Wf[:D].astype(ml_dtypes.bfloat16)    # [256, 256] att half
    w2 = Wf[D:].astype(ml_dtypes.bfloat16)    # [256, 256] val half
    iota = np.tile(np.arange(P, dtype=np.float32)[None, :], (P, 1)) \
        .astype(ml_dtypes.bfloat16)
    entf = np.asarray(ent_feats, dtype=np.float32)

    in_maps = []
    for k in range(N_CORES):
        nids = plan["node_of"][k]
        entp = np.zeros((NODES_PER_CORE, D), dtype=ml_dtypes.bfloat16)
        m = nids >= 0
        entp[m] = entf[nids[m]].astype(ml_dtypes.bfloat16)
        valrows = valf[plan["vid"][k]]                       # [NT*128, 256]
        hrelf = plan["hrel"][k].reshape(NT, P)               # [NT, 128]
        aidx = _pack16(plan["aid"][k])                       # [128, 8*NT]
        in_maps.append(dict(
            ent=entp, attfT=attfT, attf=attfb,
            valrows=np.ascontiguousarray(valrows),
            a1r=a1r, a2r=a2r, br=br, w1=w1, w2=w2, iota=iota,
            hrelf=np.ascontiguousarray(hrelf.T).astype(ml_dtypes.bfloat16),
            aidx=aidx,
        ))
    return in_maps


def kernel(attribute_triples, att_feats, val_feats, ent_feats, a_w, a_b, W):
    plan = _host_plan(attribute_triples)
    nc = build_program(plan["T"])
    in_maps = _per_core_inputs(plan, att_feats, val_feats, ent_feats,
                               a_w, a_b, W)
    trace = os.environ.get("KERNEL_TRACE", "0") == "1"
    res = _bu.run_bass_kernel_spmd(nc, in_maps, list(range(N_CORES)), trace=trace)
    if trace and res.exec_time_ns:
        print(f"HW exec time: {res.exec_time_ns} ns")
    out = np.zeros((N_ENT, D), dtype=np.float32)
    for k in range(N_CORES):
        nids = plan["node_of"][k]
        m = nids >= 0
        out[nids[m]] = res.results[k]["out"][m].astype(np.float32)
    return out
